# revision 1
# baseline (speedup 1.0000x reference)
"""Scatter-add (A.at[index].add(B)) on 8 trn2 NeuronCores.

Strategy: value-range sharding. Host sorts rows by index value and assigns
each core a contiguous range of output rows (windows of 128 values). All
floating-point work (segment summation of B rows, addition of A) happens on
device via one-hot selection matmuls; the host only permutes/pads inputs and
concatenates the per-core output slices.

Device program per 128-value window (window = 128 consecutive output rows):
  S[p, j, v] = (idx_rel[p, j] == v)     one DVE is_equal against an iota const
  psum[v, d] = sum_j S_j^T @ B_j        K PSUM-accumulated fp32 matmuls
  out[v, d]  = psum (+ A_w for heavy windows), contiguous grouped store

A-handling: windows are processed heaviest-first (host permutation). Light
windows (row count <= (K-1)*128) have >= 128 free padding slots in their B
chunks; the host places the window's 128 A rows there with idx_rel = v, so
the selection matmul adds A for free. Heavy windows (first H_CAP positions)
get A via a DVE add from a preloaded tile instead.

DMAs are grouped G=7 windows per transfer (~2.7MB) for bandwidth efficiency.

The TRN2 instruction encodings carry a limited number of semaphore waits, so
constants (index table, iota) ship in one DRAM tensor loaded by a single DMA
and the module is built via Bacc (whose compile() legalizes multi-wait
instructions).
"""

import math
import sys

import numpy as np

sys.path.insert(0, "/opt/trn_rl_repo")

N, M, D = 100000, 500000, 128
P = 128
NCORES = 8

W_GLOBAL = (N + P - 1) // P              # 782 value-windows
WPC = (W_GLOBAL + NCORES - 1) // NCORES  # 98 windows per core
W_PAD = WPC * NCORES                     # 784
N_PAD = W_PAD * P                        # 100352 output rows before trimming
G = 7                                    # windows per DMA group (98 = 7*14)
NG = WPC // G

_BUILT = {}
_LAST_RES = None


def build_bass(K, h_cap, wpc=WPC, bufs_big=5, bufs_sel=10, bufs_small=4,
               bufs_psum=8, repeats=1):
    """Build the SPMD Bass module.

    K = chunks of 128 rows per window; h_cap = number of leading (heavy)
    window positions that receive A via a DVE add instead of embedding.
    """
    from concourse import bacc, mybir, tile

    assert wpc % G == 0
    ng = wpc // G
    if K > 8:
        bufs_big = 3
    f32 = mybir.dt.float32
    f16 = mybir.dt.float16
    iota_off = wpc * K
    cw = iota_off + K * P

    nc = bacc.Bacc("TRN2", target_bir_lowering=False, debug=False)

    b_d = nc.dram_tensor(
        "b_pad", [ng, P, G, K, 2, P], f16, kind="ExternalInput"
    ).ap()
    c_d = nc.dram_tensor("consts", [P, cw], f16, kind="ExternalInput").ap()
    ah_d = nc.dram_tensor("a_heavy", [P, h_cap, P], f32, kind="ExternalInput").ap()
    out_d = nc.dram_tensor("out", [ng, P, G, P], f32, kind="ExternalOutput").ap()

    with tile.TileContext(nc) as tc:
        with (
            tc.tile_pool(name="const", bufs=1) as cpool,
            tc.tile_pool(name="big", bufs=bufs_big) as bpool,
            tc.tile_pool(name="sel", bufs=bufs_sel) as selpool,
            tc.tile_pool(name="small", bufs=bufs_small) as spool,
            tc.tile_pool(name="psum", bufs=bufs_psum, space="PSUM") as ppool,
        ):
            c_t = cpool.tile([P, cw], f16)
            nc.sync.dma_start(out=c_t[:], in_=c_d[:])
            ah_t = cpool.tile([P, h_cap, P], f32)
            nc.scalar.dma_start(out=ah_t[:], in_=ah_d[:])

            for g in range(ng * repeats):
                g = g % ng
                b_t = bpool.tile([P, G, K, 2, P], f16, tag="b")
                nc.sync.dma_start(out=b_t[:], in_=b_d[g])
                o_t = spool.tile([P, G, P], f32, tag="o")

                for u in range(G):
                    pos = g * G + u
                    s_t = selpool.tile([P, K, P], f16, tag="s")
                    nc.vector.tensor_tensor(
                        out=s_t[:],
                        in0=c_t[:, pos * K : (pos + 1) * K].to_broadcast([P, K, P]),
                        in1=c_t[:, iota_off : iota_off + K * P],
                        op=mybir.AluOpType.is_equal,
                    )
                    ps = ppool.tile([P, P], f32)
                    for j in range(K):
                        for h in range(2):
                            nc.tensor.matmul(
                                out=ps[:],
                                lhsT=s_t[:, j, :],
                                rhs=b_t[:, u, j, h, :],
                                start=(j == 0 and h == 0),
                                stop=(j == K - 1 and h == 1),
                            )
                    if pos < h_cap:
                        nc.vector.tensor_add(
                            out=o_t[:, u, :], in0=ps[:], in1=ah_t[:, pos, :]
                        )
                    else:
                        nc.scalar.copy(out=o_t[:, u, :], in_=ps[:])
                nc.scalar.dma_start(out=out_d[g], in_=o_t[:])
    nc.compile()
    return nc


def shard_inputs(index, A, B):
    """Sort rows by index value, bin into 128-value windows (heaviest-first
    per core), pad to K chunks, embed A rows in light windows' padding."""
    idx = np.asarray(index).astype(np.int64).ravel()
    A = np.asarray(A, dtype=np.float32)
    B = np.ascontiguousarray(np.asarray(B, dtype=np.float32))

    order = np.argsort(idx, kind="stable")
    sidx = idx[order]
    bounds = np.searchsorted(sidx, np.arange(0, N_PAD + 1, P)).astype(np.int64)
    counts = np.diff(bounds)                      # (W_PAD,) rows per window
    K = max(6, math.ceil(counts.max() / P)) if counts.max() > 0 else 6
    light_max = (K - 1) * P                       # max count that fits A rows

    counts_c = counts.reshape(NCORES, WPC)
    # perm[c, pos] = wloc processed at position pos (heaviest first)
    perm = np.argsort(-counts_c, axis=1, kind="stable")
    wpos = np.empty_like(perm)                    # wpos[c, wloc] = pos
    for c in range(NCORES):
        wpos[c, perm[c]] = np.arange(WPC)
    n_heavy = int((counts_c > light_max).sum(axis=1).max())
    h_cap = max(1, n_heavy)

    win = (sidx // P).astype(np.int64)
    qpos = np.arange(M, dtype=np.int64) - bounds[win]
    p = qpos % P
    j = qpos // P
    core = win // WPC
    wloc = win % WPC
    pos = wpos[core, wloc]

    # b layout: (core, group, p, wsub, j, hi/lo, d) keyed by position.
    # fp16 two-term split: hi + lo == value to ~2^-22 relative, so the pair
    # of half-rate-free fp16 matmuls reproduces the fp32 product exactly
    # enough while halving PE passes.
    b_all = np.zeros((NCORES, NG, P, G, K, 2, P), np.float16)
    b_src = B[order]
    b_hi = b_src.astype(np.float16)
    b_lo = (b_src - b_hi.astype(np.float32)).astype(np.float16)
    b_all[core, pos // G, p, pos % G, j, 0] = b_hi
    b_all[core, pos // G, p, pos % G, j, 1] = b_lo

    # consts layout: [idx table (p, pos, j) | iota]
    iota_off = WPC * K
    cw = iota_off + K * P
    consts_arr = np.full((NCORES, P, cw), -1.0, np.float16)
    consts_arr[:, :, iota_off:] = np.tile(np.arange(P, dtype=np.float16), K)
    consts_arr[core, p, pos * K + j] = (sidx - win * P).astype(np.float16)

    a_pad = np.zeros((N_PAD, D), np.float32)
    a_pad[:N] = A
    a_win = a_pad.reshape(NCORES, WPC, P, P)      # (c, wloc, v, d)

    # Embed A rows into light windows' padding (positions >= h_cap).
    ce, pe_ = np.meshgrid(np.arange(NCORES), np.arange(h_cap, WPC),
                          indexing="ij")
    ce, pe_ = ce.ravel(), pe_.ravel()             # (n_embed,) core/pos pairs
    wl = perm[ce, pe_]
    cnt = counts_c[ce, wl]
    assert (cnt <= light_max).all()
    ce3 = np.repeat(ce, P)
    pe3 = np.repeat(pe_, P)
    wl3 = np.repeat(wl, P)
    q3 = np.repeat(cnt, P) + np.tile(np.arange(P), len(ce))
    v3 = np.tile(np.arange(P), len(ce))
    a_rows = a_win[ce3, wl3, v3]
    a_hi = a_rows.astype(np.float16)
    a_lo = (a_rows - a_hi.astype(np.float32)).astype(np.float16)
    b_all[ce3, pe3 // G, q3 % P, pe3 % G, q3 // P, 0] = a_hi
    b_all[ce3, pe3 // G, q3 % P, pe3 % G, q3 // P, 1] = a_lo
    consts_arr[ce3, q3 % P, pe3 * K + q3 // P] = v3.astype(np.float32)

    # Heavy positions get A via DVE add from a preloaded tile: (c, v, pos, d)
    a_heavy = np.zeros((NCORES, P, h_cap, P), np.float32)
    hw = perm[:, :h_cap]                          # (c, h_cap) wlocs
    a_heavy[:] = a_win[np.arange(NCORES)[:, None], hw].transpose(0, 2, 1, 3)

    in_maps = [
        {"b_pad": b_all[c], "consts": consts_arr[c], "a_heavy": a_heavy[c]}
        for c in range(NCORES)
    ]
    return K, h_cap, perm, in_maps


def assemble_out(results, perm):
    """results[c]["out"] is (ng, v, wsub, d) in position order; undo the
    per-core window permutation and concatenate."""
    full = np.empty((N_PAD, D), np.float32)
    rows = full.reshape(NCORES, WPC, P, D)
    for c in range(NCORES):
        o = np.asarray(results[c]["out"]).transpose(0, 2, 1, 3)
        rows[c, perm[c]] = o.reshape(WPC, P, D)
    return full[:N]


def kernel(index, A, B):
    from concourse.bass_utils import run_bass_kernel_spmd

    K, h_cap, perm, in_maps = shard_inputs(index, A, B)
    key = (K, h_cap)
    if key not in _BUILT:
        _BUILT[key] = build_bass(K, h_cap)
    nc = _BUILT[key]

    res = run_bass_kernel_spmd(nc, in_maps, list(range(NCORES)))
    global _LAST_RES
    _LAST_RES = res
    full = assemble_out(res.results, perm)
    return np.ascontiguousarray(full.astype(np.float32))



# revision 2
# speedup vs baseline: 1.3310x; 1.3310x over previous
"""Scatter-add (A.at[index].add(B)) on 8 trn2 NeuronCores.

Strategy: value-range sharding. Host sorts rows by index value and assigns
each core a contiguous range of output rows (windows of 128 values). All
floating-point work (segment summation of B rows, addition of A) happens on
device via one-hot selection matmuls; the host only permutes/pads inputs and
concatenates the per-core output slices.

Device program per 128-value window (window = 128 consecutive output rows):
  S[p, j, v] = (idx_rel[p, j] == v)     one DVE is_equal against an iota const
  psum[v, d] = sum_j S_j^T @ B_j        K PSUM-accumulated fp32 matmuls
  out[v, d]  = psum (+ A_w for heavy windows), contiguous grouped store

All streamed data is fp16: B rows, embedded/heavy A rows, and the output
(cast fp32->fp16 on the PSUM->SBUF copy, widened to fp32 on host). The
fp32-accumulated sum of ~6 fp16-rounded terms lands at ~5e-4 scale-relative
error, far inside the 2e-2 gate, and halves HBM traffic versus fp32/hi+lo.

A-handling: windows are processed lightest-first (host permutation). Light
windows (row count <= (K-1)*128) have >= 128 free padding slots in their B
chunks; the host places the window's 128 A rows there with idx_rel = v, so
the selection matmul adds A for free. Heavy windows (last H_CAP positions)
get A via a DVE add from an fp16 tile loaded early but needed only at the
tail, so its DMA never gates the pipeline.

DMAs are grouped G=7 windows per transfer (~1.4MB) for bandwidth efficiency.

The TRN2 instruction encodings carry a limited number of semaphore waits, so
constants (index table, iota) ship in one DRAM tensor loaded by a single DMA
and the module is built via Bacc (whose compile() legalizes multi-wait
instructions).
"""

import math
import sys

import numpy as np

sys.path.insert(0, "/opt/trn_rl_repo")

N, M, D = 100000, 500000, 128
P = 128
NCORES = 8

W_GLOBAL = (N + P - 1) // P              # 782 value-windows
WPC = (W_GLOBAL + NCORES - 1) // NCORES  # 98 windows per core
W_PAD = WPC * NCORES                     # 784
N_PAD = W_PAD * P                        # 100352 output rows before trimming
G = 7                                    # windows per DMA group (98 = 7*14)
NG = WPC // G

_BUILT = {}
_LAST_RES = None


def build_bass(K, h_cap, wpc=WPC, bufs_big=6, bufs_sel=10, bufs_small=4,
               bufs_psum=8, repeats=1):
    """Build the SPMD Bass module.

    K = chunks of 128 rows per window; h_cap = number of trailing (heavy)
    window positions that receive A via a DVE add instead of embedding.
    """
    from concourse import bacc, mybir, tile

    assert wpc % G == 0
    ng = wpc // G
    f32 = mybir.dt.float32
    f16 = mybir.dt.float16
    iota_off = wpc * K
    cw = iota_off + K * P
    n_light = wpc - h_cap

    nc = bacc.Bacc("TRN2", target_bir_lowering=False, debug=False)

    b_d = nc.dram_tensor(
        "b_pad", [ng, P, G, K, P], f16, kind="ExternalInput"
    ).ap()
    c_d = nc.dram_tensor("consts", [P, cw], f16, kind="ExternalInput").ap()
    ah_d = nc.dram_tensor("a_heavy", [P, h_cap, P], f16, kind="ExternalInput").ap()
    out_d = nc.dram_tensor("out", [ng, P, G, P], f16, kind="ExternalOutput").ap()

    with tile.TileContext(nc) as tc:
        with (
            tc.tile_pool(name="const", bufs=1) as cpool,
            tc.tile_pool(name="big", bufs=bufs_big) as bpool,
            tc.tile_pool(name="sel", bufs=bufs_sel) as selpool,
            tc.tile_pool(name="small", bufs=bufs_small) as spool,
            tc.tile_pool(name="psum", bufs=bufs_psum, space="PSUM") as ppool,
        ):
            c_t = cpool.tile([P, cw], f16)
            nc.sync.dma_start(out=c_t[:], in_=c_d[:])
            ah_t = cpool.tile([P, h_cap, P], f16)

            for g in range(ng * repeats):
                g = g % ng
                b_t = bpool.tile([P, G, K, P], f16, tag="b")
                nc.sync.dma_start(out=b_t[:], in_=b_d[g])
                if g == 0:
                    # Emitted after b0 on the same (in-order) queue: the
                    # first group's transfer starts first, and ah streams
                    # during group-0 compute, long before the heavy tail.
                    nc.sync.dma_start(out=ah_t[:], in_=ah_d[:])
                o_t = spool.tile([P, G, P], f16, tag="o")

                for u in range(G):
                    pos = g * G + u
                    s_t = selpool.tile([P, K, P], f16, tag="s")
                    nc.vector.tensor_tensor(
                        out=s_t[:],
                        in0=c_t[:, pos * K : (pos + 1) * K].to_broadcast([P, K, P]),
                        in1=c_t[:, iota_off : iota_off + K * P],
                        op=mybir.AluOpType.is_equal,
                    )
                    ps = ppool.tile([P, P], f32)
                    for j in range(K):
                        nc.tensor.matmul(
                            out=ps[:],
                            lhsT=s_t[:, j, :],
                            rhs=b_t[:, u, j, :],
                            start=(j == 0),
                            stop=(j == K - 1),
                        )
                    if pos >= n_light:
                        nc.vector.tensor_tensor(
                            out=o_t[:, u, :],
                            in0=ps[:],
                            in1=ah_t[:, pos - n_light, :],
                            op=mybir.AluOpType.add,
                        )
                    else:
                        nc.scalar.copy(out=o_t[:, u, :], in_=ps[:])
                nc.scalar.dma_start(out=out_d[g], in_=o_t[:])
    nc.compile()
    return nc


def shard_inputs(index, A, B):
    """Sort rows by index value, bin into 128-value windows (lightest-first
    per core), pad to K chunks, embed A rows in light windows' padding."""
    idx = np.asarray(index).astype(np.int64).ravel()
    A = np.asarray(A, dtype=np.float32)
    B = np.ascontiguousarray(np.asarray(B, dtype=np.float32))

    order = np.argsort(idx, kind="stable")
    sidx = idx[order]
    bounds = np.searchsorted(sidx, np.arange(0, N_PAD + 1, P)).astype(np.int64)
    counts = np.diff(bounds)                      # (W_PAD,) rows per window
    K = max(6, math.ceil(counts.max() / P)) if counts.max() > 0 else 6
    light_max = (K - 1) * P                       # max count that fits A rows

    counts_c = counts.reshape(NCORES, WPC)
    # perm[c, pos] = wloc processed at position pos (lightest first)
    perm = np.argsort(counts_c, axis=1, kind="stable")
    wpos = np.empty_like(perm)                    # wpos[c, wloc] = pos
    for c in range(NCORES):
        wpos[c, perm[c]] = np.arange(WPC)
    n_heavy = int((counts_c > light_max).sum(axis=1).max())
    h_cap = max(1, n_heavy)
    n_light = WPC - h_cap

    win = (sidx // P).astype(np.int64)
    qpos = np.arange(M, dtype=np.int64) - bounds[win]
    p = qpos % P
    j = qpos // P
    core = win // WPC
    wloc = win % WPC
    pos = wpos[core, wloc]

    # b layout: (core, group, p, wsub, j, d) keyed by position, fp16.
    b_all = np.zeros((NCORES, NG, P, G, K, P), np.float16)
    b_all[core, pos // G, p, pos % G, j] = B[order].astype(np.float16)

    # consts layout: [idx table (p, pos, j) | iota]
    iota_off = WPC * K
    cw = iota_off + K * P
    consts_arr = np.full((NCORES, P, cw), -1.0, np.float16)
    consts_arr[:, :, iota_off:] = np.tile(np.arange(P, dtype=np.float16), K)
    consts_arr[core, p, pos * K + j] = (sidx - win * P).astype(np.float16)

    a_pad = np.zeros((N_PAD, D), np.float32)
    a_pad[:N] = A
    a_win = a_pad.reshape(NCORES, WPC, P, P)      # (c, wloc, v, d)

    # Embed A rows into light windows' padding (positions < n_light).
    ce, pe_ = np.meshgrid(np.arange(NCORES), np.arange(n_light),
                          indexing="ij")
    ce, pe_ = ce.ravel(), pe_.ravel()             # (n_embed,) core/pos pairs
    wl = perm[ce, pe_]
    cnt = counts_c[ce, wl]
    assert (cnt <= light_max).all()
    ce3 = np.repeat(ce, P)
    pe3 = np.repeat(pe_, P)
    wl3 = np.repeat(wl, P)
    q3 = np.repeat(cnt, P) + np.tile(np.arange(P), len(ce))
    v3 = np.tile(np.arange(P), len(ce))
    b_all[ce3, pe3 // G, q3 % P, pe3 % G, q3 // P] = (
        a_win[ce3, wl3, v3].astype(np.float16)
    )
    consts_arr[ce3, q3 % P, pe3 * K + q3 // P] = v3.astype(np.float16)

    # Heavy positions get A via DVE add from a preloaded tile: (c, v, i, d)
    a_heavy = np.zeros((NCORES, P, h_cap, P), np.float16)
    hw = perm[:, n_light:]                        # (c, h_cap) wlocs
    a_heavy[:] = (
        a_win[np.arange(NCORES)[:, None], hw].transpose(0, 2, 1, 3)
    ).astype(np.float16)

    in_maps = [
        {"b_pad": b_all[c], "consts": consts_arr[c], "a_heavy": a_heavy[c]}
        for c in range(NCORES)
    ]
    return K, h_cap, perm, in_maps


def assemble_out(results, perm):
    """results[c]["out"] is (ng, v, wsub, d) fp16 in position order; undo the
    per-core window permutation, widen to fp32, and concatenate."""
    full = np.empty((N_PAD, D), np.float32)
    rows = full.reshape(NCORES, WPC, P, D)
    for c in range(NCORES):
        o = np.asarray(results[c]["out"]).astype(np.float32)
        o = o.transpose(0, 2, 1, 3)
        rows[c, perm[c]] = o.reshape(WPC, P, D)
    return full[:N]


def kernel(index, A, B):
    from concourse.bass_utils import run_bass_kernel_spmd

    K, h_cap, perm, in_maps = shard_inputs(index, A, B)
    key = (K, h_cap)
    if key not in _BUILT:
        _BUILT[key] = build_bass(K, h_cap)
    nc = _BUILT[key]

    res = run_bass_kernel_spmd(nc, in_maps, list(range(NCORES)))
    global _LAST_RES
    _LAST_RES = res
    full = assemble_out(res.results, perm)
    return np.ascontiguousarray(full.astype(np.float32))


# revision 3
# speedup vs baseline: 1.8613x; 1.3984x over previous
"""Scatter-add (A.at[index].add(B)) on 8 trn2 NeuronCores.

Strategy: value-range sharding. Host sorts rows by index value and assigns
each core a contiguous range of output rows (windows of 128 values). All
floating-point work (segment summation of B rows, addition of A) happens on
device via one-hot selection matmuls; the host only permutes/pads inputs and
concatenates the per-core output slices.

Device program per 128-value window (window = 128 consecutive output rows):
  S[p, j, v] = (idx_rel[p, j] == v)     one DVE is_equal against an iota const
  psum[v, d] = sum_j S_j^T @ B_j        K PSUM-accumulated fp16 matmuls
  out[v, d]  = psum (+ I @ A_w for heavy windows), contiguous grouped store

All streamed data is fp16: B rows, embedded/heavy A rows, and the output
(cast fp32->fp16 on the PSUM->SBUF copy, widened to fp32 on host). The
fp32-accumulated sum of ~6 fp16-rounded terms lands at ~5e-4 scale-relative
error, far inside the 2e-2 gate, and halves HBM traffic versus fp32/hi+lo.

DVE fast path: TensorTensor only reaches the 2x perf mode when every
operand's innermost AP dim is packed 2-byte (stride 1, count >= 2). A
stride-0 broadcast of the index column disqualifies it, so the index table
stores each value TWICE and in0 reads [K, 64 (stride 0), 2 (stride 1)] —
identical semantics, packed innermost dim, half the DVE time.

A-handling: windows are processed lightest-first (host permutation). Light
windows (row count <= (K-1)*128) have >= 128 free padding slots in their B
chunks; the host places the window's 128 A rows there with idx_rel = v, so
the selection matmul adds A for free. Heavy windows (last H_CAP positions)
get A via one extra PE matmul (identity one-hot x A-chunk) from an fp16
tile loaded early but needed only at the tail.

B ships in per-position-span DMAs: a [1,2,4] prologue so the first matmul
starts ~3us earlier, steady-state spans of 7 (~1.4MB), and a [4,2,1] tail.

The TRN2 instruction encodings carry a limited number of semaphore waits, so
constants (index table, iota, identity) ship in one DRAM tensor loaded by a
single DMA and the module is built via Bacc (whose compile() legalizes
multi-wait instructions).
"""

import math
import sys

import numpy as np

sys.path.insert(0, "/opt/trn_rl_repo")

N, M, D = 100000, 500000, 128
P = 128
NCORES = 8

W_GLOBAL = (N + P - 1) // P              # 782 value-windows
WPC = (W_GLOBAL + NCORES - 1) // NCORES  # 98 windows per core
W_PAD = WPC * NCORES                     # 784
N_PAD = W_PAD * P                        # 100352 output rows before trimming
SPANS = [1, 2, 4] + [7] * 12 + [4, 2, 1]
assert sum(SPANS) == WPC
GMAX = max(SPANS)

_BUILT = {}
_LAST_RES = None


def build_bass(K, h_cap, wpc=WPC, bufs_big=6, bufs_sel=10, bufs_small=4,
               bufs_psum=8):
    """Build the SPMD Bass module.

    K = chunks of 128 rows per window; h_cap = number of trailing (heavy)
    window positions that receive A via an identity matmul instead of
    embedding.
    """
    from concourse import bacc, mybir, tile

    f32 = mybir.dt.float32
    f16 = mybir.dt.float16
    iota_off = wpc * K * 2
    id_off = iota_off + K * P
    cw = id_off + P
    n_light = wpc - h_cap

    nc = bacc.Bacc("TRN2", target_bir_lowering=False, debug=False)

    b_d = nc.dram_tensor("b_pad", [P, wpc, K, P], f16, kind="ExternalInput").ap()
    c_d = nc.dram_tensor("consts", [P, cw], f16, kind="ExternalInput").ap()
    ah_d = nc.dram_tensor("a_heavy", [P, h_cap, P], f16, kind="ExternalInput").ap()
    out_d = nc.dram_tensor("out", [P, wpc, P], f16, kind="ExternalOutput").ap()

    with tile.TileContext(nc) as tc:
        with (
            tc.tile_pool(name="const", bufs=1) as cpool,
            tc.tile_pool(name="big", bufs=bufs_big) as bpool,
            tc.tile_pool(name="sel", bufs=bufs_sel) as selpool,
            tc.tile_pool(name="small", bufs=bufs_small) as spool,
            tc.tile_pool(name="psum", bufs=bufs_psum, space="PSUM") as ppool,
        ):
            c_t = cpool.tile([P, cw], f16)
            nc.sync.dma_start(out=c_t[:], in_=c_d[:])
            ah_t = cpool.tile([P, h_cap, P], f16)

            pos0 = 0
            for gi, g in enumerate(SPANS):
                b_t = bpool.tile([P, GMAX, K, P], f16, tag="b")
                nc.sync.dma_start(
                    out=b_t[:, :g], in_=b_d[:, pos0 : pos0 + g]
                )
                if gi == 0:
                    # Emitted after the first span on the same (in-order)
                    # queue: the first window's transfer starts first, and
                    # ah streams during early compute, long before the
                    # heavy tail needs it.
                    nc.sync.dma_start(out=ah_t[:], in_=ah_d[:])
                o_t = spool.tile([P, GMAX, P], f16, tag="o")

                for u in range(g):
                    pos = pos0 + u
                    s_t = selpool.tile([P, K, P], f16, tag="s")
                    in0 = (
                        c_t[:, pos * K * 2 : (pos + 1) * K * 2]
                        .rearrange("p (k q) -> p k q", k=K)
                        .unsqueeze(2)
                        .broadcast_to([P, K, 64, 2])
                    )
                    in1 = c_t[:, iota_off:id_off].rearrange(
                        "p (k v q) -> p k v q", k=K, v=64, q=2
                    )
                    nc.vector.tensor_tensor(
                        out=s_t[:].rearrange("p k (v q) -> p k v q", v=64, q=2),
                        in0=in0,
                        in1=in1,
                        op=mybir.AluOpType.is_equal,
                    )
                    ps = ppool.tile([P, P], f32)
                    heavy = pos >= n_light
                    for j in range(K):
                        nc.tensor.matmul(
                            out=ps[:],
                            lhsT=s_t[:, j, :],
                            rhs=b_t[:, u, j, :],
                            start=(j == 0),
                            stop=(j == K - 1 and not heavy),
                        )
                    if heavy:
                        nc.tensor.matmul(
                            out=ps[:],
                            lhsT=c_t[:, id_off : id_off + P],
                            rhs=ah_t[:, pos - n_light, :],
                            start=False,
                            stop=True,
                        )
                    nc.scalar.copy(out=o_t[:, u, :], in_=ps[:])
                nc.scalar.dma_start(
                    out=out_d[:, pos0 : pos0 + g], in_=o_t[:, :g]
                )
                pos0 += g
    nc.compile()
    return nc


def shard_inputs(index, A, B):
    """Sort rows by index value, bin into 128-value windows (lightest-first
    per core), pad to K chunks, embed A rows in light windows' padding."""
    idx = np.asarray(index).astype(np.int64).ravel()
    A = np.asarray(A, dtype=np.float32)
    B = np.ascontiguousarray(np.asarray(B, dtype=np.float32))

    order = np.argsort(idx, kind="stable")
    sidx = idx[order]
    bounds = np.searchsorted(sidx, np.arange(0, N_PAD + 1, P)).astype(np.int64)
    counts = np.diff(bounds)                      # (W_PAD,) rows per window
    K = max(6, math.ceil(counts.max() / P)) if counts.max() > 0 else 6
    light_max = (K - 1) * P                       # max count that fits A rows

    counts_c = counts.reshape(NCORES, WPC)
    # perm[c, pos] = wloc processed at position pos (lightest first)
    perm = np.argsort(counts_c, axis=1, kind="stable")
    wpos = np.empty_like(perm)                    # wpos[c, wloc] = pos
    for c in range(NCORES):
        wpos[c, perm[c]] = np.arange(WPC)
    n_heavy = int((counts_c > light_max).sum(axis=1).max())
    h_cap = max(1, n_heavy)
    n_light = WPC - h_cap

    win = (sidx // P).astype(np.int64)
    qpos = np.arange(M, dtype=np.int64) - bounds[win]
    p = qpos % P
    j = qpos // P
    core = win // WPC
    wloc = win % WPC
    pos = wpos[core, wloc]

    # b layout: (core, p, pos, j, d) keyed by position, fp16.
    b_all = np.zeros((NCORES, P, WPC, K, P), np.float16)
    b_all[core, p, pos, j] = B[order].astype(np.float16)

    # consts layout: [idx pairs (p, pos, j, 2) | iota | identity]
    iota_off = WPC * K * 2
    id_off = iota_off + K * P
    cw = id_off + P
    consts_arr = np.full((NCORES, P, cw), -1.0, np.float16)
    consts_arr[:, :, iota_off:id_off] = np.tile(np.arange(P, dtype=np.float16), K)
    consts_arr[:, :, id_off:] = np.eye(P, dtype=np.float16)
    idx_rel = (sidx - win * P).astype(np.float16)
    consts_arr[core, p, (pos * K + j) * 2] = idx_rel
    consts_arr[core, p, (pos * K + j) * 2 + 1] = idx_rel

    a_pad = np.zeros((N_PAD, D), np.float32)
    a_pad[:N] = A
    a_win = a_pad.reshape(NCORES, WPC, P, P)      # (c, wloc, v, d)

    # Embed A rows into light windows' padding (positions < n_light).
    ce, pe_ = np.meshgrid(np.arange(NCORES), np.arange(n_light),
                          indexing="ij")
    ce, pe_ = ce.ravel(), pe_.ravel()             # (n_embed,) core/pos pairs
    wl = perm[ce, pe_]
    cnt = counts_c[ce, wl]
    assert (cnt <= light_max).all()
    ce3 = np.repeat(ce, P)
    pe3 = np.repeat(pe_, P)
    wl3 = np.repeat(wl, P)
    q3 = np.repeat(cnt, P) + np.tile(np.arange(P), len(ce))
    v3 = np.tile(np.arange(P), len(ce))
    b_all[ce3, q3 % P, pe3, q3 // P] = a_win[ce3, wl3, v3].astype(np.float16)
    v16 = v3.astype(np.float16)
    consts_arr[ce3, q3 % P, (pe3 * K + q3 // P) * 2] = v16
    consts_arr[ce3, q3 % P, (pe3 * K + q3 // P) * 2 + 1] = v16

    # Heavy positions get A via an identity matmul from a preloaded tile:
    # (c, v, i, d) with partition = A-row index within the window.
    a_heavy = np.zeros((NCORES, P, h_cap, P), np.float16)
    hw = perm[:, n_light:]                        # (c, h_cap) wlocs
    a_heavy[:] = (
        a_win[np.arange(NCORES)[:, None], hw].transpose(0, 2, 1, 3)
    ).astype(np.float16)

    in_maps = [
        {"b_pad": b_all[c], "consts": consts_arr[c], "a_heavy": a_heavy[c]}
        for c in range(NCORES)
    ]
    return K, h_cap, perm, in_maps


def assemble_out(results, perm):
    """results[c]["out"] is (v, pos, d) fp16 in position order; undo the
    per-core window permutation, widen to fp32, and concatenate."""
    full = np.empty((N_PAD, D), np.float32)
    rows = full.reshape(NCORES, WPC, P, D)
    for c in range(NCORES):
        o = np.asarray(results[c]["out"]).astype(np.float32)
        rows[c, perm[c]] = o.transpose(1, 0, 2)
    return full[:N]


def kernel(index, A, B):
    from concourse.bass_utils import run_bass_kernel_spmd

    K, h_cap, perm, in_maps = shard_inputs(index, A, B)
    key = (K, h_cap)
    if key not in _BUILT:
        _BUILT[key] = build_bass(K, h_cap)
    nc = _BUILT[key]

    res = run_bass_kernel_spmd(nc, in_maps, list(range(NCORES)))
    global _LAST_RES
    _LAST_RES = res
    full = assemble_out(res.results, perm)
    return np.ascontiguousarray(full.astype(np.float32))
